# revision 5
# baseline (speedup 1.0000x reference)
"""GroupSort (pairwise channel sort) Trainium2 Bass kernel.

out[:, 2k]   = min(x[:, 2k], x[:, 2k+1])
out[:, 2k+1] = max(x[:, 2k], x[:, 2k+1])

x: [32, 512, 56, 56] f32.  Batch-sharded across 8 NeuronCores (4 per core).
Per core the shard [4, 512, 56, 56] is viewed as [1024, 6272]: each row is
one (batch, channel-pair) - first 3136 cols = even channel's H*W pixels,
last 3136 = odd channel's.  Memory-bound: 25.7 MB in + 25.7 MB out per core.

Compute is replicated bit-exactly from the reference:
  z = relu(xe - xo); out_e = xe - z; out_o = xo + z
DVE does the three tensor_tensor ops (f32 1x mode), ACT does the relu.
Outputs overwrite the input tile, so each tile needs one full-row store
(25 KiB descriptors) instead of two half-row ones.

SDMA engine 15 sustains only ~22.4 GB/s vs ~26.3 for engines 0-14
(measured), and DMA descriptors are assigned round-robin by descriptor
index starting at engine 0 (verified by probe).  A [128, N] tile hands
every engine 8 descriptors and the whole kernel finishes when engine 15
does.  Instead: 8 main tiles of 127 rows (engine 15 gets 7 descriptors
per DMA, engines 0-14 get 8) plus one 8-row remainder tile whose
descriptors go to engines 0-7.  Per-engine time: eng0-7 65 rows,
eng8-14 64 rows, eng15 56 rows ~ 125 us each at measured rates.
"""

import os
import sys

import numpy as np

sys.path.insert(0, "/opt/trn_rl_repo")

import concourse.tile as tile
from concourse import bacc, mybir
from concourse.bass_utils import run_bass_kernel_spmd

def _install_trace_shim():
    """The image's antenv package lacks axon_hooks, which
    run_bass_kernel_spmd imports for trace=True. Install the same
    ctypes-based NTFF hook trn_boot would have registered, and keep
    profile artifacts local instead of uploading to a bucket."""
    try:
        import types as _types

        from concourse import bass_utils as _bu

        _bu.upload_artifacts = lambda tmpdir: tmpdir
        if "antenv.axon_hooks" not in sys.modules:
            from trn_agent_boot.trn_boot import _ntff_profile_via_ctypes

            _hook = _ntff_profile_via_ctypes("/opt/axon/libaxon_pjrt.so")
            _mod = _types.ModuleType("antenv.axon_hooks")
            _mod.get_axon_ntff_profile_hook = lambda: _hook
            _mod.set_axon_ntff_profile_hook = lambda h: None
            sys.modules["antenv.axon_hooks"] = _mod
    except Exception:
        pass


N_CORES = 8
B, C, H, W = 32, 512, 56, 56
HW = H * W  # 3136
B_PER = B // N_CORES  # 4
ROWS = B_PER * C // 2  # 1024 pair-rows per core
COLS = 2 * HW  # 6272
P = 128
N_TILES = ROWS // P  # 8

_cache = {}


def _build_nc():
    nc = bacc.Bacc(
        "TRN2", debug=False, num_devices=N_CORES, enable_partition_id=False
    )
    x = nc.dram_tensor("x", [ROWS, COLS], mybir.dt.float32, kind="ExternalInput").ap()
    o = nc.dram_tensor(
        "out", [ROWS, COLS], mybir.dt.float32, kind="ExternalOutput"
    ).ap()

    relu = mybir.ActivationFunctionType.Relu
    PM = 127  # main-tile rows; engine 15 gets 7 of 127 descriptors
    REM = ROWS - N_TILES * PM  # 8 remainder rows -> engines 0-7

    def compute(it, zt, p):
        """z = relu(xe - xo); out_e = xe - z; out_o = xo + z (in place)."""
        nc.vector.tensor_sub(zt[0:p, :], it[0:p, 0:HW], it[0:p, HW:COLS])
        nc.scalar.activation(zt[0:p, :], zt[0:p, :], relu)
        nc.vector.tensor_sub(it[0:p, 0:HW], it[0:p, 0:HW], zt[0:p, :])
        nc.vector.tensor_add(it[0:p, HW:COLS], it[0:p, HW:COLS], zt[0:p, :])

    with tile.TileContext(nc, num_cores=N_CORES) as tc:
        with (
            tc.tile_pool(name="inp", bufs=4) as inp,
            tc.tile_pool(name="zp", bufs=3) as zp,
            tc.tile_pool(name="remp", bufs=1) as remp,
        ):
            # Remainder rows first: tiny, fills the pipeline-fill dead time.
            rem = remp.tile([REM, COLS], mybir.dt.float32)
            nc.sync.dma_start(out=rem[:], in_=x[N_TILES * PM : ROWS, :])
            zr = zp.tile([P, HW], mybir.dt.float32)
            compute(rem, zr, REM)
            nc.scalar.dma_start(out=o[N_TILES * PM : ROWS, :], in_=rem[:])
            for t in range(N_TILES):
                r = t * PM
                it = inp.tile([P, COLS], mybir.dt.float32)
                nc.sync.dma_start(out=it[0:PM, :], in_=x[r : r + PM, :])
                zt = zp.tile([P, HW], mybir.dt.float32)
                compute(it, zt, PM)
                # one full-row store (25 KiB contiguous per partition)
                nc.scalar.dma_start(out=o[r : r + PM, :], in_=it[0:PM, :])
    nc.compile()
    return nc


def _get_nc():
    if "nc" not in _cache:
        _cache["nc"] = _build_nc()
    return _cache["nc"]


def kernel(
    x: np.ndarray,
    _trace: bool = False,
    _tmpdir: str | None = None,
    _trace_cores: list | None = None,
):
    assert x.shape == (B, C, H, W), x.shape
    x = np.ascontiguousarray(x, dtype=np.float32)
    shards = x.reshape(N_CORES, ROWS, COLS)
    in_maps = [{"x": shards[i]} for i in range(N_CORES)]

    nc = _get_nc()
    if _trace:
        _install_trace_shim()
        os.environ.pop("BASS_NEVER_TRACE", None)
    else:
        # run_bass_kernel_spmd also enables tracing when BASS_TRACE is set
        # in the environment; keep the grading path deterministic.
        os.environ["BASS_NEVER_TRACE"] = "1"
    res = run_bass_kernel_spmd(
        nc,
        in_maps,
        list(range(N_CORES)),
        trace=_trace,
        tmpdir=_tmpdir,
        trace_cores=_trace_cores,
    )
    out = np.empty((N_CORES, ROWS, COLS), dtype=np.float32)
    for i in range(N_CORES):
        out[i] = res.results[i]["out"]
    if _trace:
        kernel.last_exec_time_ns = res.exec_time_ns
        kernel.last_results = res
    return out.reshape(B, C, H, W)


if __name__ == "__main__":
    rng = np.random.default_rng(0)
    xt = rng.standard_normal((B, C, H, W), dtype=np.float32)
    yt = kernel(xt)
    xe, xo = xt[:, 0::2], xt[:, 1::2]
    z = np.maximum(xe - xo, 0)
    exp = np.empty_like(xt)
    exp[:, 0::2] = xe - z
    exp[:, 1::2] = xo + z
    err = np.abs(yt - exp).max()
    print("absmax err:", err)


# revision 7
# speedup vs baseline: 11.4217x; 11.4217x over previous
"""GroupSort (pairwise channel sort) Trainium2 Bass kernel.

out[:, 2k]   = min(x[:, 2k], x[:, 2k+1])
out[:, 2k+1] = max(x[:, 2k], x[:, 2k+1])

x: [32, 512, 56, 56] f32.  Batch-sharded across 8 NeuronCores (4 per core).
Per core the shard [4, 512, 56, 56] is viewed as [1024, 6272]: each row is
one (batch, channel-pair) - first 3136 cols = even channel's H*W pixels,
last 3136 = odd channel's.  Memory-bound: 25.7 MB in + 25.7 MB out per core.

Compute is replicated bit-exactly from the reference:
  z = relu(xe - xo); out_e = xe - z; out_o = xo + z
DVE does the three tensor_tensor ops (f32 1x mode), ACT does the relu.
Outputs overwrite the input tile, so each tile needs one full-row store
(25 KiB descriptors) instead of two half-row ones.

SDMA engine 15 sustains only ~22.4 GB/s vs ~26.3 for engines 0-14
(measured), and a DMA's descriptors are split over the largest queue
count <= 16 that divides the partition count (verified by probe: 128
partitions -> 16 queues x 8, 120 -> 15 queues x 8 on engines 0-14,
127 -> ONE engine).  So: 6 x [128-row] + 1 x [16-row] tiles spread
evenly over all 16 engines, plus 2 x [120-row] tiles that land only on
engines 0-14.  Per-engine rows: eng0-14 get 65, eng15 gets 49 -- both
finish in ~124 us at measured rates instead of eng15 dragging to 145.
"""

import os
import sys

import numpy as np

sys.path.insert(0, "/opt/trn_rl_repo")

import concourse.tile as tile
from concourse import bacc, mybir
from concourse.bass_utils import run_bass_kernel_spmd

def _install_trace_shim():
    """The image's antenv package lacks axon_hooks, which
    run_bass_kernel_spmd imports for trace=True. Install the same
    ctypes-based NTFF hook trn_boot would have registered, and keep
    profile artifacts local instead of uploading to a bucket."""
    try:
        import types as _types

        from concourse import bass_utils as _bu

        _bu.upload_artifacts = lambda tmpdir: tmpdir
        if "antenv.axon_hooks" not in sys.modules:
            from trn_agent_boot.trn_boot import _ntff_profile_via_ctypes

            _hook = _ntff_profile_via_ctypes("/opt/axon/libaxon_pjrt.so")
            _mod = _types.ModuleType("antenv.axon_hooks")
            _mod.get_axon_ntff_profile_hook = lambda: _hook
            _mod.set_axon_ntff_profile_hook = lambda h: None
            sys.modules["antenv.axon_hooks"] = _mod
    except Exception:
        pass


N_CORES = 8
B, C, H, W = 32, 512, 56, 56
HW = H * W  # 3136
B_PER = B // N_CORES  # 4
ROWS = B_PER * C // 2  # 1024 pair-rows per core
COLS = 2 * HW  # 6272
P = 128
N_TILES = ROWS // P  # 8

_cache = {}


def _build_nc():
    nc = bacc.Bacc(
        "TRN2", debug=False, num_devices=N_CORES, enable_partition_id=False
    )
    x = nc.dram_tensor("x", [ROWS, COLS], mybir.dt.float32, kind="ExternalInput").ap()
    o = nc.dram_tensor(
        "out", [ROWS, COLS], mybir.dt.float32, kind="ExternalOutput"
    ).ap()

    relu = mybir.ActivationFunctionType.Relu
    # Tile row counts: 16-row starter, 2x120 (engines 0-14 only), 6x128.
    TILE_ROWS = [16, 120, 120] + [P] * 6
    assert sum(TILE_ROWS) == ROWS

    with tile.TileContext(nc, num_cores=N_CORES) as tc:
        with (
            tc.tile_pool(name="inp", bufs=5) as inp,
            tc.tile_pool(name="zp", bufs=3) as zp,
        ):
            r = 0
            for rows in TILE_ROWS:
                it = inp.tile([P, COLS], mybir.dt.float32)
                nc.sync.dma_start(out=it[0:rows, :], in_=x[r : r + rows, :])
                zt = zp.tile([P, HW], mybir.dt.float32)
                # z = relu(xe - xo); out_e = xe - z; out_o = xo + z (in place)
                nc.vector.tensor_sub(
                    zt[0:rows, :], it[0:rows, 0:HW], it[0:rows, HW:COLS]
                )
                nc.scalar.activation(zt[0:rows, :], zt[0:rows, :], relu)
                nc.vector.tensor_sub(
                    it[0:rows, 0:HW], it[0:rows, 0:HW], zt[0:rows, :]
                )
                nc.vector.tensor_add(
                    it[0:rows, HW:COLS], it[0:rows, HW:COLS], zt[0:rows, :]
                )
                # one full-row store (25 KiB contiguous per partition)
                nc.scalar.dma_start(out=o[r : r + rows, :], in_=it[0:rows, :])
                r += rows
    nc.compile()
    return nc


def _get_nc():
    if "nc" not in _cache:
        _cache["nc"] = _build_nc()
    return _cache["nc"]


def kernel(
    x: np.ndarray,
    _trace: bool = False,
    _tmpdir: str | None = None,
    _trace_cores: list | None = None,
):
    assert x.shape == (B, C, H, W), x.shape
    x = np.ascontiguousarray(x, dtype=np.float32)
    shards = x.reshape(N_CORES, ROWS, COLS)
    in_maps = [{"x": shards[i]} for i in range(N_CORES)]

    nc = _get_nc()
    if _trace:
        _install_trace_shim()
        os.environ.pop("BASS_NEVER_TRACE", None)
    else:
        # run_bass_kernel_spmd also enables tracing when BASS_TRACE is set
        # in the environment; keep the grading path deterministic.
        os.environ["BASS_NEVER_TRACE"] = "1"
    res = run_bass_kernel_spmd(
        nc,
        in_maps,
        list(range(N_CORES)),
        trace=_trace,
        tmpdir=_tmpdir,
        trace_cores=_trace_cores,
    )
    out = np.empty((N_CORES, ROWS, COLS), dtype=np.float32)
    for i in range(N_CORES):
        out[i] = res.results[i]["out"]
    if _trace:
        kernel.last_exec_time_ns = res.exec_time_ns
        kernel.last_results = res
    return out.reshape(B, C, H, W)


if __name__ == "__main__":
    rng = np.random.default_rng(0)
    xt = rng.standard_normal((B, C, H, W), dtype=np.float32)
    yt = kernel(xt)
    xe, xo = xt[:, 0::2], xt[:, 1::2]
    z = np.maximum(xe - xo, 0)
    exp = np.empty_like(xt)
    exp[:, 0::2] = xe - z
    exp[:, 1::2] = xo + z
    err = np.abs(yt - exp).max()
    print("absmax err:", err)


# revision 9
# speedup vs baseline: 11.5180x; 1.0084x over previous
"""GroupSort (pairwise channel sort) Trainium2 Bass kernel.

out[:, 2k]   = min(x[:, 2k], x[:, 2k+1])
out[:, 2k+1] = max(x[:, 2k], x[:, 2k+1])

x: [32, 512, 56, 56] f32.  Batch-sharded across 8 NeuronCores (4 per core).
Per core the shard [4, 512, 56, 56] is viewed as [1024, 6272]: each row is
one (batch, channel-pair) - first 3136 cols = even channel's H*W pixels,
last 3136 = odd channel's.  Memory-bound: 25.7 MB in + 25.7 MB out per core.

Compute is replicated bit-exactly from the reference:
  z = relu(xe - xo); out_e = xe - z; out_o = xo + z
DVE does the three tensor_tensor ops (f32 1x mode), ACT does the relu.
Outputs overwrite the input tile, so each tile needs one full-row store
(25 KiB descriptors) instead of two half-row ones.

SDMA engine 15 sustains only ~22.4 GB/s vs ~26.3 for engines 0-14
(measured), and a DMA's descriptors are split over the largest queue
count <= 16 that divides the partition count (verified by probe: 128
partitions -> 16 queues x 8, 120 -> 15 queues x 8 on engines 0-14,
127 -> ONE engine).  Measured caveat: 15-way-split LOADS run ~2x slow
per descriptor (SBUF-write port crossing?), while 15-way STORES run at
line rate.  So all loads are [128]-row tiles, and the rebalance is
store-side only: the last two tiles store as [120 rows] (engines 0-14)
+ [8 rows] (engines 0-7).  Engine 15 ends up with 112 descriptors vs
130 for engines 0-7 -- every engine finishes in ~125 us instead of
engine 15 dragging the uniform layout to ~145 us.
"""

import os
import sys

import numpy as np

sys.path.insert(0, "/opt/trn_rl_repo")

import concourse.tile as tile
from concourse import bacc, mybir
from concourse.bass_utils import run_bass_kernel_spmd

def _install_trace_shim():
    """The image's antenv package lacks axon_hooks, which
    run_bass_kernel_spmd imports for trace=True. Install the same
    ctypes-based NTFF hook trn_boot would have registered, and keep
    profile artifacts local instead of uploading to a bucket."""
    try:
        import types as _types

        from concourse import bass_utils as _bu

        _bu.upload_artifacts = lambda tmpdir: tmpdir
        if "antenv.axon_hooks" not in sys.modules:
            from trn_agent_boot.trn_boot import _ntff_profile_via_ctypes

            _hook = _ntff_profile_via_ctypes("/opt/axon/libaxon_pjrt.so")
            _mod = _types.ModuleType("antenv.axon_hooks")
            _mod.get_axon_ntff_profile_hook = lambda: _hook
            _mod.set_axon_ntff_profile_hook = lambda h: None
            sys.modules["antenv.axon_hooks"] = _mod
    except Exception:
        pass


N_CORES = 8
B, C, H, W = 32, 512, 56, 56
HW = H * W  # 3136
B_PER = B // N_CORES  # 4
ROWS = B_PER * C // 2  # 1024 pair-rows per core
COLS = 2 * HW  # 6272
P = 128
N_TILES = ROWS // P  # 8

_cache = {}


def _build_nc():
    nc = bacc.Bacc(
        "TRN2", debug=False, num_devices=N_CORES, enable_partition_id=False
    )
    x = nc.dram_tensor("x", [ROWS, COLS], mybir.dt.float32, kind="ExternalInput").ap()
    o = nc.dram_tensor(
        "out", [ROWS, COLS], mybir.dt.float32, kind="ExternalOutput"
    ).ap()

    relu = mybir.ActivationFunctionType.Relu
    N_SPLIT = 2  # tiles whose stores skip engine 15 (last N_SPLIT tiles)

    with tile.TileContext(nc, num_cores=N_CORES) as tc:
        with (
            tc.tile_pool(name="inp", bufs=5) as inp,
            tc.tile_pool(name="zp", bufs=3) as zp,
        ):
            for t in range(N_TILES):
                r = t * P
                it = inp.tile([P, COLS], mybir.dt.float32)
                nc.sync.dma_start(out=it[:], in_=x[r : r + P, :])
                zt = zp.tile([P, HW], mybir.dt.float32)
                # z = relu(xe - xo); out_e = xe - z; out_o = xo + z (in place)
                nc.vector.tensor_sub(zt[:], it[:, 0:HW], it[:, HW:COLS])
                nc.scalar.activation(zt[:], zt[:], relu)
                nc.vector.tensor_sub(it[:, 0:HW], it[:, 0:HW], zt[:])
                nc.vector.tensor_add(it[:, HW:COLS], it[:, HW:COLS], zt[:])
                # full-row stores (25 KiB contiguous per partition)
                if t >= N_TILES - N_SPLIT:
                    # split store: [120] -> engines 0-14, [8] -> engines 0-7
                    nc.scalar.dma_start(out=o[r : r + 120, :], in_=it[0:120, :])
                    nc.scalar.dma_start(out=o[r + 120 : r + P, :], in_=it[120:P, :])
                else:
                    nc.scalar.dma_start(out=o[r : r + P, :], in_=it[:])
    nc.compile()
    return nc


def _get_nc():
    if "nc" not in _cache:
        _cache["nc"] = _build_nc()
    return _cache["nc"]


def kernel(
    x: np.ndarray,
    _trace: bool = False,
    _tmpdir: str | None = None,
    _trace_cores: list | None = None,
):
    assert x.shape == (B, C, H, W), x.shape
    x = np.ascontiguousarray(x, dtype=np.float32)
    shards = x.reshape(N_CORES, ROWS, COLS)
    in_maps = [{"x": shards[i]} for i in range(N_CORES)]

    nc = _get_nc()
    if _trace:
        _install_trace_shim()
        os.environ.pop("BASS_NEVER_TRACE", None)
    else:
        # run_bass_kernel_spmd also enables tracing when BASS_TRACE is set
        # in the environment; keep the grading path deterministic.
        os.environ["BASS_NEVER_TRACE"] = "1"
    res = run_bass_kernel_spmd(
        nc,
        in_maps,
        list(range(N_CORES)),
        trace=_trace,
        tmpdir=_tmpdir,
        trace_cores=_trace_cores,
    )
    out = np.empty((N_CORES, ROWS, COLS), dtype=np.float32)
    for i in range(N_CORES):
        out[i] = res.results[i]["out"]
    if _trace:
        kernel.last_exec_time_ns = res.exec_time_ns
        kernel.last_results = res
    return out.reshape(B, C, H, W)


if __name__ == "__main__":
    rng = np.random.default_rng(0)
    xt = rng.standard_normal((B, C, H, W), dtype=np.float32)
    yt = kernel(xt)
    xe, xo = xt[:, 0::2], xt[:, 1::2]
    z = np.maximum(xe - xo, 0)
    exp = np.empty_like(xt)
    exp[:, 0::2] = xe - z
    exp[:, 1::2] = xo + z
    err = np.abs(yt - exp).max()
    print("absmax err:", err)


# revision 11
# speedup vs baseline: 12.0573x; 1.0468x over previous
"""GroupSort (pairwise channel sort) Trainium2 Bass kernel.

out[:, 2k]   = min(x[:, 2k], x[:, 2k+1])
out[:, 2k+1] = max(x[:, 2k], x[:, 2k+1])

x: [32, 512, 56, 56] f32.  Batch-sharded across 8 NeuronCores (4 per core).
Per core the shard [4, 512, 56, 56] is viewed as [1024, 6272]: each row is
one (batch, channel-pair) - first 3136 cols = even channel's H*W pixels,
last 3136 = odd channel's.  Memory-bound: 25.7 MB in + 25.7 MB out per core.

Compute is replicated bit-exactly from the reference:
  z = relu(xe - xo); out_e = xe - z; out_o = xo + z
DVE does the three tensor_tensor ops (f32 1x mode), ACT does the relu.
Outputs overwrite the input tile, so each tile needs one full-row store
(25 KiB descriptors) instead of two half-row ones.

SDMA engine 15 sustains only ~22.4 GB/s vs ~26.3 for engines 0-14
(measured), and a DMA's descriptors are split over the largest queue
count <= 16 that divides the partition count (verified by probe: 128
partitions -> 16 queues x 8, 120 -> 15 queues x 8 on engines 0-14,
127 -> ONE engine).  Measured caveat: 15-way-split LOADS run ~2x slow
per descriptor (SBUF-write port crossing?), while 15-way STORES run at
line rate.  So all loads are [128]-row tiles, and the rebalance is
store-side only: the last two tiles store as [120 rows] (engines 0-14)
+ [8 rows] (engines 0-7).  Engine 15 ends up with 112 descriptors vs
130 for engines 0-7 -- every engine finishes in ~125 us instead of
engine 15 dragging the uniform layout to ~145 us.
"""

import os
import sys

import numpy as np

sys.path.insert(0, "/opt/trn_rl_repo")

import concourse.tile as tile
from concourse import bacc, mybir
from concourse.bass_utils import run_bass_kernel_spmd

def _install_trace_shim():
    """The image's antenv package lacks axon_hooks, which
    run_bass_kernel_spmd imports for trace=True. Install the same
    ctypes-based NTFF hook trn_boot would have registered, and keep
    profile artifacts local instead of uploading to a bucket."""
    try:
        import types as _types

        from concourse import bass_utils as _bu

        _bu.upload_artifacts = lambda tmpdir: tmpdir
        if "antenv.axon_hooks" not in sys.modules:
            from trn_agent_boot.trn_boot import _ntff_profile_via_ctypes

            _hook = _ntff_profile_via_ctypes("/opt/axon/libaxon_pjrt.so")
            _mod = _types.ModuleType("antenv.axon_hooks")
            _mod.get_axon_ntff_profile_hook = lambda: _hook
            _mod.set_axon_ntff_profile_hook = lambda h: None
            sys.modules["antenv.axon_hooks"] = _mod
    except Exception:
        pass


N_CORES = 8
B, C, H, W = 32, 512, 56, 56
HW = H * W  # 3136
B_PER = B // N_CORES  # 4
ROWS = B_PER * C // 2  # 1024 pair-rows per core
COLS = 2 * HW  # 6272
P = 128
N_TILES = ROWS // P  # 8

_cache = {}


def _build_nc():
    nc = bacc.Bacc(
        "TRN2", debug=False, num_devices=N_CORES, enable_partition_id=False
    )
    x = nc.dram_tensor("x", [ROWS, COLS], mybir.dt.float32, kind="ExternalInput").ap()
    o = nc.dram_tensor(
        "out", [ROWS, COLS], mybir.dt.float32, kind="ExternalOutput"
    ).ap()

    relu = mybir.ActivationFunctionType.Relu
    # Tiles whose stores skip engine 15.  Placed FIRST: 15-way-split
    # stores only run at line rate while the load queue is also active
    # (measured); at the tail they run ~2x slow, so the tail stays [128].
    N_SPLIT = 2

    with tile.TileContext(nc, num_cores=N_CORES) as tc:
        with (
            tc.tile_pool(name="inp", bufs=5) as inp,
            tc.tile_pool(name="zp", bufs=3) as zp,
        ):
            for t in range(N_TILES):
                r = t * P
                it = inp.tile([P, COLS], mybir.dt.float32)
                nc.sync.dma_start(out=it[:], in_=x[r : r + P, :])
                zt = zp.tile([P, HW], mybir.dt.float32)
                # z = relu(xe - xo); out_e = xe - z; out_o = xo + z (in place)
                nc.vector.tensor_sub(zt[:], it[:, 0:HW], it[:, HW:COLS])
                nc.scalar.activation(zt[:], zt[:], relu)
                nc.vector.tensor_sub(it[:, 0:HW], it[:, 0:HW], zt[:])
                nc.vector.tensor_add(it[:, HW:COLS], it[:, HW:COLS], zt[:])
                # full-row stores (25 KiB contiguous per partition)
                if t < N_SPLIT:
                    # split store: [120] -> engines 0-14, [8] -> engines 0-7
                    nc.scalar.dma_start(out=o[r : r + 120, :], in_=it[0:120, :])
                    nc.scalar.dma_start(out=o[r + 120 : r + P, :], in_=it[120:P, :])
                else:
                    nc.scalar.dma_start(out=o[r : r + P, :], in_=it[:])
    nc.compile()
    return nc


def _get_nc():
    if "nc" not in _cache:
        _cache["nc"] = _build_nc()
    return _cache["nc"]


def kernel(
    x: np.ndarray,
    _trace: bool = False,
    _tmpdir: str | None = None,
    _trace_cores: list | None = None,
):
    assert x.shape == (B, C, H, W), x.shape
    x = np.ascontiguousarray(x, dtype=np.float32)
    shards = x.reshape(N_CORES, ROWS, COLS)
    in_maps = [{"x": shards[i]} for i in range(N_CORES)]

    nc = _get_nc()
    if _trace:
        _install_trace_shim()
        os.environ.pop("BASS_NEVER_TRACE", None)
    else:
        # run_bass_kernel_spmd also enables tracing when BASS_TRACE is set
        # in the environment; keep the grading path deterministic.
        os.environ["BASS_NEVER_TRACE"] = "1"
    res = run_bass_kernel_spmd(
        nc,
        in_maps,
        list(range(N_CORES)),
        trace=_trace,
        tmpdir=_tmpdir,
        trace_cores=_trace_cores,
    )
    out = np.empty((N_CORES, ROWS, COLS), dtype=np.float32)
    for i in range(N_CORES):
        out[i] = res.results[i]["out"]
    if _trace:
        kernel.last_exec_time_ns = res.exec_time_ns
        kernel.last_results = res
    return out.reshape(B, C, H, W)


if __name__ == "__main__":
    rng = np.random.default_rng(0)
    xt = rng.standard_normal((B, C, H, W), dtype=np.float32)
    yt = kernel(xt)
    xe, xo = xt[:, 0::2], xt[:, 1::2]
    z = np.maximum(xe - xo, 0)
    exp = np.empty_like(xt)
    exp[:, 0::2] = xe - z
    exp[:, 1::2] = xo + z
    err = np.abs(yt - exp).max()
    print("absmax err:", err)


# revision 12
# speedup vs baseline: 12.7289x; 1.0557x over previous
"""GroupSort (pairwise channel sort) Trainium2 Bass kernel.

out[:, 2k]   = min(x[:, 2k], x[:, 2k+1])
out[:, 2k+1] = max(x[:, 2k], x[:, 2k+1])

x: [32, 512, 56, 56] f32.  Batch-sharded across 8 NeuronCores (4 per core).
Per core the shard [4, 512, 56, 56] is viewed as [1024, 6272]: each row is
one (batch, channel-pair) - first 3136 cols = even channel's H*W pixels,
last 3136 = odd channel's.  Memory-bound: 25.7 MB in + 25.7 MB out per core.

Compute is replicated bit-exactly from the reference:
  z = relu(xe - xo); out_e = xe - z; out_o = xo + z
DVE does the three tensor_tensor ops (f32 1x mode), ACT does the relu.
Outputs overwrite the input tile, so each tile needs one full-row store
(25 KiB descriptors) instead of two half-row ones.

SDMA engine 15 sustains only ~22.4 GB/s vs ~26.3 for engines 0-14
(measured), and a DMA's descriptors are split over the largest queue
count <= 16 that divides the partition count (verified by probe: 128
partitions -> 16 queues x 8, 120 -> 15 queues x 8 on engines 0-14,
127 -> ONE engine).  Measured caveat: 15-way-split LOADS run ~2x slow
per descriptor (SBUF-write port crossing?), while 15-way STORES run at
line rate.  So all loads are [128]-row tiles, and the rebalance is
store-side only: the last two tiles store as [120 rows] (engines 0-14)
+ [8 rows] (engines 0-7).  Engine 15 ends up with 112 descriptors vs
130 for engines 0-7 -- every engine finishes in ~125 us instead of
engine 15 dragging the uniform layout to ~145 us.
"""

import os
import sys

import numpy as np

sys.path.insert(0, "/opt/trn_rl_repo")

import concourse.tile as tile
from concourse import bacc, mybir
from concourse.bass_utils import run_bass_kernel_spmd

def _install_trace_shim():
    """The image's antenv package lacks axon_hooks, which
    run_bass_kernel_spmd imports for trace=True. Install the same
    ctypes-based NTFF hook trn_boot would have registered, and keep
    profile artifacts local instead of uploading to a bucket."""
    try:
        import types as _types

        from concourse import bass_utils as _bu

        _bu.upload_artifacts = lambda tmpdir: tmpdir
        if "antenv.axon_hooks" not in sys.modules:
            from trn_agent_boot.trn_boot import _ntff_profile_via_ctypes

            _hook = _ntff_profile_via_ctypes("/opt/axon/libaxon_pjrt.so")
            _mod = _types.ModuleType("antenv.axon_hooks")
            _mod.get_axon_ntff_profile_hook = lambda: _hook
            _mod.set_axon_ntff_profile_hook = lambda h: None
            sys.modules["antenv.axon_hooks"] = _mod
    except Exception:
        pass


N_CORES = 8
B, C, H, W = 32, 512, 56, 56
HW = H * W  # 3136
B_PER = B // N_CORES  # 4
ROWS = B_PER * C // 2  # 1024 pair-rows per core
COLS = 2 * HW  # 6272
P = 128
N_TILES = ROWS // P  # 8

_cache = {}


def _build_nc():
    nc = bacc.Bacc(
        "TRN2", debug=False, num_devices=N_CORES, enable_partition_id=False
    )
    x = nc.dram_tensor("x", [ROWS, COLS], mybir.dt.float32, kind="ExternalInput").ap()
    o = nc.dram_tensor(
        "out", [ROWS, COLS], mybir.dt.float32, kind="ExternalOutput"
    ).ap()

    relu = mybir.ActivationFunctionType.Relu
    # Tiles whose stores skip engine 15.  Placed FIRST: 15-way-split
    # stores only run at line rate while the load queue is also active
    # (measured); at the tail they run ~2x slow, so the tail stays [128].
    N_SPLIT = 2

    with tile.TileContext(nc, num_cores=N_CORES) as tc:
        with (
            tc.tile_pool(name="inp", bufs=5) as inp,
            tc.tile_pool(name="zp", bufs=3) as zp,
        ):
            # Software-pipelined emission: v(t+1) on DVE and relu(t+1) on
            # ACT are issued BEFORE oute/outo(t) and store(t), so the ACT
            # stream never blocks relu(t+1) behind store(t)'s sem wait and
            # the DVE conveyor (3 TT ops/tile) never stalls on ACT.
            its, zts = [], []

            def stage_front(t):
                r = t * P
                it = inp.tile([P, COLS], mybir.dt.float32)
                nc.sync.dma_start(out=it[:], in_=x[r : r + P, :])
                zt = zp.tile([P, HW], mybir.dt.float32)
                nc.vector.tensor_sub(zt[:], it[:, 0:HW], it[:, HW:COLS])
                nc.scalar.activation(zt[:], zt[:], relu)
                its.append(it)
                zts.append(zt)

            stage_front(0)
            for t in range(N_TILES):
                if t + 1 < N_TILES:
                    stage_front(t + 1)
                it, zt = its[t], zts[t]
                r = t * P
                nc.vector.tensor_sub(it[:, 0:HW], it[:, 0:HW], zt[:])
                nc.vector.tensor_add(it[:, HW:COLS], it[:, HW:COLS], zt[:])
                # full-row stores (25 KiB contiguous per partition)
                if t < N_SPLIT:
                    # split store: [120] -> engines 0-14, [8] -> engines 0-7
                    nc.scalar.dma_start(out=o[r : r + 120, :], in_=it[0:120, :])
                    nc.scalar.dma_start(out=o[r + 120 : r + P, :], in_=it[120:P, :])
                else:
                    nc.scalar.dma_start(out=o[r : r + P, :], in_=it[:])
    nc.compile()
    return nc


def _get_nc():
    if "nc" not in _cache:
        _cache["nc"] = _build_nc()
    return _cache["nc"]


def kernel(
    x: np.ndarray,
    _trace: bool = False,
    _tmpdir: str | None = None,
    _trace_cores: list | None = None,
):
    assert x.shape == (B, C, H, W), x.shape
    x = np.ascontiguousarray(x, dtype=np.float32)
    shards = x.reshape(N_CORES, ROWS, COLS)
    in_maps = [{"x": shards[i]} for i in range(N_CORES)]

    nc = _get_nc()
    if _trace:
        _install_trace_shim()
        os.environ.pop("BASS_NEVER_TRACE", None)
    else:
        # run_bass_kernel_spmd also enables tracing when BASS_TRACE is set
        # in the environment; keep the grading path deterministic.
        os.environ["BASS_NEVER_TRACE"] = "1"
    res = run_bass_kernel_spmd(
        nc,
        in_maps,
        list(range(N_CORES)),
        trace=_trace,
        tmpdir=_tmpdir,
        trace_cores=_trace_cores,
    )
    out = np.empty((N_CORES, ROWS, COLS), dtype=np.float32)
    for i in range(N_CORES):
        out[i] = res.results[i]["out"]
    if _trace:
        kernel.last_exec_time_ns = res.exec_time_ns
        kernel.last_results = res
    return out.reshape(B, C, H, W)


if __name__ == "__main__":
    rng = np.random.default_rng(0)
    xt = rng.standard_normal((B, C, H, W), dtype=np.float32)
    yt = kernel(xt)
    xe, xo = xt[:, 0::2], xt[:, 1::2]
    z = np.maximum(xe - xo, 0)
    exp = np.empty_like(xt)
    exp[:, 0::2] = xe - z
    exp[:, 1::2] = xo + z
    err = np.abs(yt - exp).max()
    print("absmax err:", err)


# revision 14
# speedup vs baseline: 13.3402x; 1.0480x over previous
"""GroupSort (pairwise channel sort) Trainium2 Bass kernel.

out[:, 2k]   = min(x[:, 2k], x[:, 2k+1])
out[:, 2k+1] = max(x[:, 2k], x[:, 2k+1])

x: [32, 512, 56, 56] f32.  Batch-sharded across 8 NeuronCores (4 per core).
Per core the shard [4, 512, 56, 56] is viewed as [1024, 6272]: each row is
one (batch, channel-pair) - first 3136 cols = even channel's H*W pixels,
last 3136 = odd channel's.  Memory-bound: 25.7 MB in + 25.7 MB out per core.

Compute is replicated bit-exactly from the reference:
  z = relu(xe - xo); out_e = xe - z; out_o = xo + z
DVE does the three tensor_tensor ops (f32 1x mode), ACT does the relu.
Outputs overwrite the input tile, so each tile needs one full-row store
(25 KiB descriptors) instead of two half-row ones.

SDMA engine 15 sustains only ~22.4 GB/s vs ~26.3 for engines 0-14
(measured), and a DMA's descriptors are split over the largest queue
count <= 16 that divides the partition count (verified by probe: 128
partitions -> 16 queues x 8, 120 -> 15 queues x 8 on engines 0-14,
127 -> ONE engine).  Measured caveat: 15-way-split LOADS run ~2x slow
per descriptor (SBUF-write port crossing?), while 15-way STORES run at
line rate.  So all loads are [128]-row tiles, and the rebalance is
store-side only: the last two tiles store as [120 rows] (engines 0-14)
+ [8 rows] (engines 0-7).  Engine 15 ends up with 112 descriptors vs
130 for engines 0-7 -- every engine finishes in ~125 us instead of
engine 15 dragging the uniform layout to ~145 us.
"""

import os
import sys

import numpy as np

sys.path.insert(0, "/opt/trn_rl_repo")

import concourse.tile as tile
from concourse import bacc, mybir
from concourse.bass_utils import run_bass_kernel_spmd

def _install_trace_shim():
    """The image's antenv package lacks axon_hooks, which
    run_bass_kernel_spmd imports for trace=True. Install the same
    ctypes-based NTFF hook trn_boot would have registered, and keep
    profile artifacts local instead of uploading to a bucket."""
    try:
        import types as _types

        from concourse import bass_utils as _bu

        _bu.upload_artifacts = lambda tmpdir: tmpdir
        if "antenv.axon_hooks" not in sys.modules:
            from trn_agent_boot.trn_boot import _ntff_profile_via_ctypes

            _hook = _ntff_profile_via_ctypes("/opt/axon/libaxon_pjrt.so")
            _mod = _types.ModuleType("antenv.axon_hooks")
            _mod.get_axon_ntff_profile_hook = lambda: _hook
            _mod.set_axon_ntff_profile_hook = lambda h: None
            sys.modules["antenv.axon_hooks"] = _mod
    except Exception:
        pass


N_CORES = 8
B, C, H, W = 32, 512, 56, 56
HW = H * W  # 3136
B_PER = B // N_CORES  # 4
ROWS = B_PER * C // 2  # 1024 pair-rows per core
COLS = 2 * HW  # 6272
P = 128
N_TILES = ROWS // P  # 8

_cache = {}


def _build_nc():
    nc = bacc.Bacc(
        "TRN2", debug=False, num_devices=N_CORES, enable_partition_id=False
    )
    x = nc.dram_tensor("x", [ROWS, COLS], mybir.dt.float32, kind="ExternalInput").ap()
    o = nc.dram_tensor(
        "out", [ROWS, COLS], mybir.dt.float32, kind="ExternalOutput"
    ).ap()

    relu = mybir.ActivationFunctionType.Relu
    # Tiles whose stores skip engine 15.  Placed FIRST: 15-way-split
    # stores only run at line rate while the load queue is also active
    # (measured); at the tail they run ~2x slow, so the tail stays [128].
    N_SPLIT = 2

    HHW = HW // 2  # column half

    with tile.TileContext(nc, num_cores=N_CORES) as tc:
        with (
            tc.tile_pool(name="inp", bufs=6) as inp,
            tc.tile_pool(name="zp", bufs=3) as zp,
        ):
            # Software-pipelined emission: v(t+1) on DVE and relu(t+1) on
            # ACT are issued BEFORE oute/outo(t) and store(t), so the ACT
            # stream never blocks relu(t+1) behind store(t)'s sem wait and
            # the DVE conveyor (3 TT ops/tile) never stalls on ACT.
            # z is computed in column halves so the z tile is half-sized,
            # freeing SBUF for a 6th input buffer.
            units = [(t, h) for t in range(N_TILES) for h in (0, 1)]
            its, zts = {}, {}

            def stage_front(u):
                t, h = units[u]
                if h == 0:
                    it = inp.tile([P, COLS], mybir.dt.float32)
                    nc.sync.dma_start(out=it[:], in_=x[t * P : (t + 1) * P, :])
                    its[t] = it
                it = its[t]
                a, b = h * HHW, (h + 1) * HHW
                zt = zp.tile([P, HHW], mybir.dt.float32)
                nc.vector.tensor_sub(zt[:], it[:, a:b], it[:, HW + a : HW + b])
                nc.scalar.activation(zt[:], zt[:], relu)
                zts[u] = zt

            stage_front(0)
            for u in range(len(units)):
                if u + 1 < len(units):
                    stage_front(u + 1)
                t, h = units[u]
                it, zt = its[t], zts[u]
                a, b = h * HHW, (h + 1) * HHW
                nc.vector.tensor_sub(it[:, a:b], it[:, a:b], zt[:])
                nc.vector.tensor_add(
                    it[:, HW + a : HW + b], it[:, HW + a : HW + b], zt[:]
                )
                if h == 1:
                    # full-row stores (25 KiB contiguous per partition)
                    r = t * P
                    if t < N_SPLIT:
                        # split store: [120] -> eng 0-14, [8] -> eng 0-7
                        nc.scalar.dma_start(
                            out=o[r : r + 120, :], in_=it[0:120, :]
                        )
                        nc.scalar.dma_start(
                            out=o[r + 120 : r + P, :], in_=it[120:P, :]
                        )
                    else:
                        nc.scalar.dma_start(out=o[r : r + P, :], in_=it[:])
    nc.compile()
    return nc


def _get_nc():
    if "nc" not in _cache:
        _cache["nc"] = _build_nc()
    return _cache["nc"]


def kernel(
    x: np.ndarray,
    _trace: bool = False,
    _tmpdir: str | None = None,
    _trace_cores: list | None = None,
):
    assert x.shape == (B, C, H, W), x.shape
    x = np.ascontiguousarray(x, dtype=np.float32)
    shards = x.reshape(N_CORES, ROWS, COLS)
    in_maps = [{"x": shards[i]} for i in range(N_CORES)]

    nc = _get_nc()
    if _trace:
        _install_trace_shim()
        os.environ.pop("BASS_NEVER_TRACE", None)
    else:
        # run_bass_kernel_spmd also enables tracing when BASS_TRACE is set
        # in the environment; keep the grading path deterministic.
        os.environ["BASS_NEVER_TRACE"] = "1"
    res = run_bass_kernel_spmd(
        nc,
        in_maps,
        list(range(N_CORES)),
        trace=_trace,
        tmpdir=_tmpdir,
        trace_cores=_trace_cores,
    )
    out = np.empty((N_CORES, ROWS, COLS), dtype=np.float32)
    for i in range(N_CORES):
        out[i] = res.results[i]["out"]
    if _trace:
        kernel.last_exec_time_ns = res.exec_time_ns
        kernel.last_results = res
    return out.reshape(B, C, H, W)


if __name__ == "__main__":
    rng = np.random.default_rng(0)
    xt = rng.standard_normal((B, C, H, W), dtype=np.float32)
    yt = kernel(xt)
    xe, xo = xt[:, 0::2], xt[:, 1::2]
    z = np.maximum(xe - xo, 0)
    exp = np.empty_like(xt)
    exp[:, 0::2] = xe - z
    exp[:, 1::2] = xo + z
    err = np.abs(yt - exp).max()
    print("absmax err:", err)


# revision 16
# speedup vs baseline: 13.6025x; 1.0197x over previous
"""GroupSort (pairwise channel sort) Trainium2 Bass kernel.

out[:, 2k]   = min(x[:, 2k], x[:, 2k+1])
out[:, 2k+1] = max(x[:, 2k], x[:, 2k+1])

x: [32, 512, 56, 56] f32.  Batch-sharded across 8 NeuronCores (4 per core).
Per core the shard [4, 512, 56, 56] is viewed as [1024, 6272]: each row is
one (batch, channel-pair) - first 3136 cols = even channel's H*W pixels,
last 3136 = odd channel's.  Memory-bound: 25.7 MB in + 25.7 MB out per core.

Compute is replicated bit-exactly from the reference:
  z = relu(xe - xo); out_e = xe - z; out_o = xo + z
DVE does the three tensor_tensor ops (f32 1x mode), ACT does the relu.
Outputs overwrite the input tile, so each tile needs one full-row store
(25 KiB descriptors) instead of two half-row ones.

SDMA engine 15 sustains only ~22.4 GB/s vs ~26.3 for engines 0-14
(measured), and a DMA's descriptors are split over the largest queue
count <= 16 that divides the partition count (verified by probe: 128
partitions -> 16 queues x 8, 120 -> 15 queues x 8 on engines 0-14,
127 -> ONE engine).  Measured caveat: 15-way-split LOADS run ~2x slow
per descriptor (SBUF-write port crossing?), while 15-way STORES run at
line rate.  So all loads are [128]-row tiles, and the rebalance is
store-side only: the last two tiles store as [120 rows] (engines 0-14)
+ [8 rows] (engines 0-7).  Engine 15 ends up with 112 descriptors vs
130 for engines 0-7 -- every engine finishes in ~125 us instead of
engine 15 dragging the uniform layout to ~145 us.
"""

import os
import sys

import numpy as np

sys.path.insert(0, "/opt/trn_rl_repo")

import concourse.tile as tile
from concourse import bacc, mybir
from concourse.bass_utils import run_bass_kernel_spmd

def _install_trace_shim():
    """The image's antenv package lacks axon_hooks, which
    run_bass_kernel_spmd imports for trace=True. Install the same
    ctypes-based NTFF hook trn_boot would have registered, and keep
    profile artifacts local instead of uploading to a bucket."""
    try:
        import types as _types

        from concourse import bass_utils as _bu

        _bu.upload_artifacts = lambda tmpdir: tmpdir
        if "antenv.axon_hooks" not in sys.modules:
            from trn_agent_boot.trn_boot import _ntff_profile_via_ctypes

            _hook = _ntff_profile_via_ctypes("/opt/axon/libaxon_pjrt.so")
            _mod = _types.ModuleType("antenv.axon_hooks")
            _mod.get_axon_ntff_profile_hook = lambda: _hook
            _mod.set_axon_ntff_profile_hook = lambda h: None
            sys.modules["antenv.axon_hooks"] = _mod
    except Exception:
        pass


N_CORES = 8
B, C, H, W = 32, 512, 56, 56
HW = H * W  # 3136
B_PER = B // N_CORES  # 4
ROWS = B_PER * C // 2  # 1024 pair-rows per core
COLS = 2 * HW  # 6272
P = 128
N_TILES = ROWS // P  # 8

_cache = {}


def _build_nc():
    nc = bacc.Bacc(
        "TRN2",
        debug=False,
        num_devices=N_CORES,
        enable_partition_id=False,
        # We issue no SWDGE (gpsimd) DMAs, so the 16 KiB descriptor-ring
        # scratch can shrink -- frees SBUF for a 7th input buffer.
        dynamic_dma_scratch_size=2048,
    )
    x = nc.dram_tensor("x", [ROWS, COLS], mybir.dt.float32, kind="ExternalInput").ap()
    o = nc.dram_tensor(
        "out", [ROWS, COLS], mybir.dt.float32, kind="ExternalOutput"
    ).ap()

    relu = mybir.ActivationFunctionType.Relu
    # Tiles whose stores skip engine 15.  Placed FIRST: 15-way-split
    # stores only run at line rate while the load queue is also active
    # (measured); at the tail they run ~2x slow, so the tail stays [128].
    N_SPLIT = 2

    HHW = HW // 2  # column half

    with tile.TileContext(nc, num_cores=N_CORES) as tc:
        with (
            tc.tile_pool(name="inp", bufs=7) as inp,
            tc.tile_pool(name="zp", bufs=2) as zp,
        ):
            # Software-pipelined emission: v(t+1) on DVE and relu(t+1) on
            # ACT are issued BEFORE oute/outo(t) and store(t), so the ACT
            # stream never blocks relu(t+1) behind store(t)'s sem wait and
            # the DVE conveyor (3 TT ops/tile) never stalls on ACT.
            # z is computed in column halves so the z tile is half-sized,
            # freeing SBUF for a 6th input buffer.
            units = [(t, h) for t in range(N_TILES) for h in (0, 1)]
            its, zts = {}, {}

            def stage_front(u):
                t, h = units[u]
                if h == 0:
                    it = inp.tile([P, COLS], mybir.dt.float32)
                    nc.sync.dma_start(out=it[:], in_=x[t * P : (t + 1) * P, :])
                    its[t] = it
                it = its[t]
                a, b = h * HHW, (h + 1) * HHW
                zt = zp.tile([P, HHW], mybir.dt.float32)
                nc.vector.tensor_sub(zt[:], it[:, a:b], it[:, HW + a : HW + b])
                nc.scalar.activation(zt[:], zt[:], relu)
                zts[u] = zt

            stage_front(0)
            for u in range(len(units)):
                if u + 1 < len(units):
                    stage_front(u + 1)
                t, h = units[u]
                it, zt = its[t], zts[u]
                a, b = h * HHW, (h + 1) * HHW
                nc.vector.tensor_sub(it[:, a:b], it[:, a:b], zt[:])
                nc.vector.tensor_add(
                    it[:, HW + a : HW + b], it[:, HW + a : HW + b], zt[:]
                )
                if h == 1:
                    # full-row stores (25 KiB contiguous per partition)
                    r = t * P
                    if t < N_SPLIT:
                        # split store: [120] -> eng 0-14, [8] -> eng 0-7
                        nc.scalar.dma_start(
                            out=o[r : r + 120, :], in_=it[0:120, :]
                        )
                        nc.scalar.dma_start(
                            out=o[r + 120 : r + P, :], in_=it[120:P, :]
                        )
                    else:
                        nc.scalar.dma_start(out=o[r : r + P, :], in_=it[:])
    nc.compile()
    return nc


def _get_nc():
    if "nc" not in _cache:
        _cache["nc"] = _build_nc()
    return _cache["nc"]


def kernel(
    x: np.ndarray,
    _trace: bool = False,
    _tmpdir: str | None = None,
    _trace_cores: list | None = None,
):
    assert x.shape == (B, C, H, W), x.shape
    x = np.ascontiguousarray(x, dtype=np.float32)
    shards = x.reshape(N_CORES, ROWS, COLS)
    in_maps = [{"x": shards[i]} for i in range(N_CORES)]

    nc = _get_nc()
    if _trace:
        _install_trace_shim()
        os.environ.pop("BASS_NEVER_TRACE", None)
    else:
        # run_bass_kernel_spmd also enables tracing when BASS_TRACE is set
        # in the environment; keep the grading path deterministic.
        os.environ["BASS_NEVER_TRACE"] = "1"
    res = run_bass_kernel_spmd(
        nc,
        in_maps,
        list(range(N_CORES)),
        trace=_trace,
        tmpdir=_tmpdir,
        trace_cores=_trace_cores,
    )
    out = np.empty((N_CORES, ROWS, COLS), dtype=np.float32)
    for i in range(N_CORES):
        out[i] = res.results[i]["out"]
    if _trace:
        kernel.last_exec_time_ns = res.exec_time_ns
        kernel.last_results = res
    return out.reshape(B, C, H, W)


if __name__ == "__main__":
    rng = np.random.default_rng(0)
    xt = rng.standard_normal((B, C, H, W), dtype=np.float32)
    yt = kernel(xt)
    xe, xo = xt[:, 0::2], xt[:, 1::2]
    z = np.maximum(xe - xo, 0)
    exp = np.empty_like(xt)
    exp[:, 0::2] = xe - z
    exp[:, 1::2] = xo + z
    err = np.abs(yt - exp).max()
    print("absmax err:", err)
